# revision 4
# baseline (speedup 1.0000x reference)
"""MoE layer (top-2 routing, 8 experts) on 8 Trainium2 NeuronCores.

Sharding: token-parallel. Each core takes a contiguous shard of 1024 tokens
(of T=8192) and runs them through all 8 experts densely, weighting each
expert's output by the on-device-computed combine weights (softmax over the
top-2 router logits; zero elsewhere). No cross-core communication is needed:
each core produces its own 1024-row slice of the output, and the host only
concatenates the slices.

Router runs in fp32 (top-2 selection must match the fp32 reference exactly;
measured logit margins ~5.7e-5 far exceed fp32 matmul rounding). FFN matmuls
run in bf16 with fp32 PSUM accumulation.
"""

import sys, os

for _p in ("/root/.axon_site", "/root/.axon_site/_ro/trn_rl_repo",
           "/root/.axon_site/_ro/pypackages", "/opt/trn_rl_repo"):
    if os.path.isdir(_p) and _p not in sys.path:
        sys.path.append(_p)

import numpy as np
import ml_dtypes

BF16 = ml_dtypes.bfloat16

T, DIM, E, K, H = 8192, 1024, 8, 2, 4096
N_CORES = 8
TPC = T // N_CORES          # tokens per core = 1024
NTB = TPC // 128            # 8 token tiles per core
ND = DIM // 128             # 8 d-chunks
NH = H // 128               # 32 h-chunks
NT = 512                    # matmul moving-dim (tokens per psum group)
NNT = TPC // NT             # 2 groups per core

_compiled = None


def _build():
    from concourse import bass, bacc, tile, mybir
    from concourse.masks import make_identity

    dt = mybir.dt
    nc = bacc.Bacc("TRN2", target_bir_lowering=False, debug=False,
                   num_devices=N_CORES)

    xtf = nc.dram_tensor("xtf", [NTB, ND, 128, 128], dt.float32, kind="ExternalInput").ap()
    xbt = nc.dram_tensor("xbt", [NTB, ND, 128, 128], dt.bfloat16, kind="ExternalInput").ap()
    wr = nc.dram_tensor("wr", [DIM, E], dt.float32, kind="ExternalInput").ap()
    w1d = nc.dram_tensor("w1d", [E, NH, 128, ND, 128], dt.bfloat16, kind="ExternalInput").ap()
    w2d = nc.dram_tensor("w2d", [E, ND, 128, NH, 128], dt.bfloat16, kind="ExternalInput").ap()
    b1d = nc.dram_tensor("b1d", [128, E, NH], dt.float32, kind="ExternalInput").ap()
    b2d = nc.dram_tensor("b2d", [128, E, ND], dt.float32, kind="ExternalInput").ap()
    out = nc.dram_tensor("out_shard", [TPC, DIM], dt.float32, kind="ExternalOutput").ap()

    with tile.TileContext(nc) as tc:
        with tc.tile_pool(name="const", bufs=1) as const, \
             tc.tile_pool(name="resident", bufs=1) as res, \
             tc.tile_pool(name="w1p", bufs=3) as w1p, \
             tc.tile_pool(name="w2p", bufs=2) as w2p, \
             tc.tile_pool(name="xrp", bufs=3) as xrp, \
             tc.tile_pool(name="vec", bufs=2) as vec, \
             tc.tile_pool(name="pmm", bufs=4, space="PSUM") as pmm, \
             tc.tile_pool(name="ptr", bufs=2, space="PSUM") as ptr:

            ident = const.tile([128, 128], dt.float32)
            make_identity(nc, ident[:])

            # resident tensors
            xall = res.tile([128, ND, TPC], dt.bfloat16)   # x^T bf16, 16KB/part
            acc = res.tile([128, NTB, DIM], dt.float32)    # output accum, 32KB/part
            hT = res.tile([128, NH, TPC], dt.bfloat16)     # hidden, 64KB/part
            cmb = res.tile([128, NTB, E], dt.float32)      # combine weights
            lg = res.tile([128, NTB, E], dt.float32)       # logits
            mx = res.tile([128, NTB, 8], dt.float32)       # sorted top-8
            wr_sb = const.tile([128, ND, E], dt.float32)
            b1sb = const.tile([128, E, NH], dt.float32)
            b2sb = const.tile([128, E, ND], dt.float32)

            nc.vector.memset(acc[:], 0.0)
            nc.sync.dma_start(b1sb[:], b1d[:])
            nc.sync.dma_start(b2sb[:], b2d[:])
            for dc in range(ND):
                nc.sync.dma_start(wr_sb[:, dc, :], wr[dc * 128:(dc + 1) * 128, :])
            # load x^T bf16 (FFN rhs), resident
            for tb in range(NTB):
                for dc in range(ND):
                    nc.sync.dma_start(xall[:, dc, tb * 128:(tb + 1) * 128],
                                      xbt[tb, dc])

            # ---- router (fp32) ----
            for tb in range(NTB):
                xt = xrp.tile([128, ND, 128], dt.float32, tag="xt")
                nc.sync.dma_start(xt[:], xtf[tb].transpose([1, 0, 2]))
                ps = ptr.tile([128, E], dt.float32, name=f"psr_{tb}", tag="psr")
                for dc in range(ND):
                    nc.tensor.matmul(ps[:], lhsT=xt[:, dc, :], rhs=wr_sb[:, dc, :],
                                     start=(dc == 0), stop=(dc == ND - 1))
                nc.scalar.copy(lg[:, tb, :], ps[:])
                nc.vector.max(mx[:, tb, :], lg[:, tb, :])

            l1 = mx[:, :, 0]                       # [128, NTB] strided
            l2 = mx[:, :, 1]
            d12 = vec.tile([128, NTB], dt.float32)
            p1 = vec.tile([128, NTB], dt.float32)
            nc.vector.tensor_sub(d12[:], l1, l2)
            nc.scalar.activation(p1[:], d12[:],
                                 bass.mybir.ActivationFunctionType.Sigmoid)
            # cmb[:, tb, e] = (lg==l1)*p1 + (lg==l2)*(1-p1)
            m1 = vec.tile([128, NTB, E], dt.float32)
            m2 = vec.tile([128, NTB, E], dt.float32)
            l1b = l1.unsqueeze(2).to_broadcast([128, NTB, E])
            l2b = l2.unsqueeze(2).to_broadcast([128, NTB, E])
            p1b = p1[:].unsqueeze(2).to_broadcast([128, NTB, E])
            nc.vector.tensor_tensor(m1[:], lg[:], l1b, mybir.AluOpType.is_equal)
            nc.vector.tensor_tensor(m2[:], lg[:], l2b, mybir.AluOpType.is_equal)
            nc.vector.tensor_tensor(m1[:], m1[:], p1b, mybir.AluOpType.mult)
            t2 = vec.tile([128, NTB, E], dt.float32)
            nc.vector.tensor_tensor(t2[:], m2[:], p1b, mybir.AluOpType.mult)
            nc.vector.tensor_sub(m2[:], m2[:], t2[:])
            nc.vector.tensor_add(cmb[:], m1[:], m2[:])

            # ---- FFN over experts ----
            for e in range(E):
                # layer 1: hT[h, t] = gelu(W1^T x^T + b1)
                for hc in range(NH):
                    w1t = w1p.tile([128, ND, 128], dt.bfloat16, tag="w1t")
                    nc.sync.dma_start(w1t[:], w1d[e, hc])
                    for nt in range(NNT):
                        ps = pmm.tile([128, NT], dt.float32, name=f"ps1_{e}_{hc}_{nt}", tag="ps")
                        sl = slice(nt * NT, (nt + 1) * NT)
                        for dc in range(ND):
                            nc.tensor.matmul(ps[:], lhsT=w1t[:, dc, :],
                                             rhs=xall[:, dc, sl],
                                             start=(dc == 0), stop=(dc == ND - 1))
                        nc.scalar.activation(hT[:, hc, sl], ps[:],
                                             bass.mybir.ActivationFunctionType.Gelu,
                                             bias=b1sb[:, e, hc:hc + 1])
                # layer 2 + transpose + weighted accumulate
                for dc in range(ND):
                    w2t = w2p.tile([128, NH, 128], dt.bfloat16, tag="w2t")
                    nc.sync.dma_start(w2t[:], w2d[e, dc])
                    for nt in range(NNT):
                        ps = pmm.tile([128, NT], dt.float32, name=f"ps2_{e}_{dc}_{nt}", tag="ps")
                        sl = slice(nt * NT, (nt + 1) * NT)
                        for hc in range(NH):
                            nc.tensor.matmul(ps[:], lhsT=w2t[:, hc, :],
                                             rhs=hT[:, hc, sl],
                                             start=(hc == 0), stop=(hc == NH - 1))
                        yt = vec.tile([128, NT], dt.float32, tag="yt")
                        nc.scalar.activation(yt[:], ps[:],
                                             bass.mybir.ActivationFunctionType.Identity,
                                             bias=b2sb[:, e, dc:dc + 1])
                        for tc_ in range(NT // 128):
                            tb = nt * (NT // 128) + tc_
                            pt = ptr.tile([128, 128], dt.float32,
                                          name=f"pt_{e}_{dc}_{nt}_{tc_}", tag="pt")
                            nc.tensor.transpose(
                                pt[:], yt[:, tc_ * 128:(tc_ + 1) * 128], ident[:])
                            a_sl = acc[:, tb, dc * 128:(dc + 1) * 128]
                            nc.vector.scalar_tensor_tensor(
                                a_sl, pt[:], cmb[:, tb, e:e + 1], a_sl,
                                op0=mybir.AluOpType.mult,
                                op1=mybir.AluOpType.add)

            for tb in range(NTB):
                nc.sync.dma_start(out[tb * 128:(tb + 1) * 128, :], acc[:, tb, :])

    nc.compile()
    return nc


def _prep_inputs(x, Wr, W1, b1, W2, b2):
    x = np.ascontiguousarray(np.asarray(x, np.float32)).reshape(T, DIM)
    Wr = np.ascontiguousarray(np.asarray(Wr, np.float32))
    W1 = np.asarray(W1, np.float32)
    b1 = np.asarray(b1, np.float32)
    W2 = np.asarray(W2, np.float32)
    b2 = np.asarray(b2, np.float32)

    w1d = np.ascontiguousarray(
        W1.astype(BF16).reshape(E, ND, 128, NH, 128).transpose(0, 3, 2, 1, 4))
    w2d = np.ascontiguousarray(
        W2.astype(BF16).reshape(E, NH, 128, ND, 128).transpose(0, 3, 2, 1, 4))
    b1d = np.ascontiguousarray(b1.reshape(E, NH, 128).transpose(2, 0, 1))
    b2d = np.ascontiguousarray(b2.reshape(E, ND, 128).transpose(2, 0, 1))

    in_maps = []
    for c in range(N_CORES):
        xs = x[c * TPC:(c + 1) * TPC]                      # [1024, 1024]
        # [NTB, ND, 128 d, 128 t] tiles of x^T
        xt = np.ascontiguousarray(
            xs.reshape(NTB, 128, ND, 128).transpose(0, 2, 3, 1))
        in_maps.append({
            "xtf": xt,
            "xbt": xt.astype(BF16),
            "wr": Wr,
            "w1d": w1d,
            "w2d": w2d,
            "b1d": b1d,
            "b2d": b2d,
        })
    return in_maps


def kernel(x, Wr, W1, b1, W2, b2, _profile=None):
    global _compiled
    from concourse.bass_utils import run_bass_kernel_spmd

    if _compiled is None:
        _compiled = _build()
    nc = _compiled
    in_maps = _prep_inputs(x, Wr, W1, b1, W2, b2)
    kwargs = {}
    if _profile:
        kwargs = dict(trace=True, tmpdir=_profile)
    res = run_bass_kernel_spmd(nc, in_maps, core_ids=list(range(N_CORES)), **kwargs)
    shards = [res.results[c]["out_shard"] for c in range(N_CORES)]
    full = np.concatenate(shards, axis=0).reshape(4, 2048, DIM).astype(np.float32)
    if _profile:
        return full, res
    return full


# revision 6
# speedup vs baseline: 1.0241x; 1.0241x over previous
"""MoE layer (top-2 routing, 8 experts) on 8 Trainium2 NeuronCores.

Sharding: token-parallel. Each core takes a contiguous shard of 1024 tokens
(of T=8192) and runs them through all 8 experts densely, weighting each
expert's output by the on-device-computed combine weights (softmax over the
top-2 router logits; zero elsewhere). No cross-core communication is needed:
each core produces its own 1024-row slice of the output, and the host only
concatenates the slices.

Router runs in fp32 (top-2 selection must match the fp32 reference exactly;
measured logit margins ~5.7e-5 far exceed fp32 matmul rounding). FFN matmuls
run in bf16 with fp32 PSUM accumulation.
"""

import sys, os

for _p in ("/root/.axon_site", "/root/.axon_site/_ro/trn_rl_repo",
           "/root/.axon_site/_ro/pypackages", "/opt/trn_rl_repo"):
    if os.path.isdir(_p) and _p not in sys.path:
        sys.path.append(_p)

import numpy as np
import ml_dtypes

BF16 = ml_dtypes.bfloat16

T, DIM, E, K, H = 8192, 1024, 8, 2, 4096
N_CORES = 8
TPC = T // N_CORES          # tokens per core = 1024
NTB = TPC // 128            # 8 token tiles per core
ND = DIM // 128             # 8 d-chunks
NH = H // 128               # 32 h-chunks
NT = 512                    # matmul moving-dim (tokens per psum group)
NNT = TPC // NT             # 2 groups per core

_compiled = None


def _build():
    from concourse import bass, bacc, tile, mybir
    from concourse.masks import make_identity

    dt = mybir.dt
    nc = bacc.Bacc("TRN2", target_bir_lowering=False, debug=False,
                   num_devices=N_CORES)

    xtf = nc.dram_tensor("xtf", [NTB, ND, 128, 128], dt.float32, kind="ExternalInput").ap()
    xbt = nc.dram_tensor("xbt", [NTB, ND, 128, 128], dt.bfloat16, kind="ExternalInput").ap()
    wr = nc.dram_tensor("wr", [DIM, E], dt.float32, kind="ExternalInput").ap()
    w1d = nc.dram_tensor("w1d", [E, NH, 128, ND, 128], dt.bfloat16, kind="ExternalInput").ap()
    w2d = nc.dram_tensor("w2d", [E, ND, 128, NH, 128], dt.bfloat16, kind="ExternalInput").ap()
    b1d = nc.dram_tensor("b1d", [128, E, NH], dt.float32, kind="ExternalInput").ap()
    b2d = nc.dram_tensor("b2d", [128, E, ND], dt.float32, kind="ExternalInput").ap()
    out = nc.dram_tensor("out_shard", [TPC, DIM], dt.float32, kind="ExternalOutput").ap()

    with tile.TileContext(nc) as tc:
        with tc.tile_pool(name="const", bufs=1) as const, \
             tc.tile_pool(name="resident", bufs=1) as res, \
             tc.tile_pool(name="w1p", bufs=3) as w1p, \
             tc.tile_pool(name="w2p", bufs=2) as w2p, \
             tc.tile_pool(name="xrp", bufs=3) as xrp, \
             tc.tile_pool(name="vec", bufs=2) as vec, \
             tc.tile_pool(name="pmm", bufs=4, space="PSUM") as pmm, \
             tc.tile_pool(name="ptr", bufs=2, space="PSUM") as ptr:

            ident = const.tile([128, 128], dt.float32)
            make_identity(nc, ident[:])
            identb = const.tile([128, 128], dt.bfloat16)
            nc.vector.tensor_copy(identb[:], ident[:])

            # resident tensors
            xall = res.tile([128, ND, TPC], dt.bfloat16)   # x^T bf16, 16KB/part
            acc = res.tile([128, NTB, DIM], dt.float32)    # output accum, 32KB/part
            hT = res.tile([128, NH, TPC], dt.bfloat16)     # hidden, 64KB/part
            cmb = res.tile([128, NTB, E], dt.float32)      # combine weights
            lg = res.tile([128, NTB, E], dt.float32)       # logits
            mx = res.tile([128, NTB, 8], dt.float32)       # sorted top-8
            wr_sb = const.tile([128, ND, E], dt.float32)
            b1sb = const.tile([128, E, NH], dt.float32)
            b2sb = const.tile([128, E, ND], dt.float32)

            nc.vector.memset(acc[:], 0.0)
            nc.sync.dma_start(b1sb[:], b1d[:])
            nc.sync.dma_start(b2sb[:], b2d[:])
            for dc in range(ND):
                nc.sync.dma_start(wr_sb[:, dc, :], wr[dc * 128:(dc + 1) * 128, :])
            # load x^T bf16 (FFN rhs), resident
            for tb in range(NTB):
                for dc in range(ND):
                    nc.sync.dma_start(xall[:, dc, tb * 128:(tb + 1) * 128],
                                      xbt[tb, dc])

            # ---- router (fp32) ----
            def emit_router():
              for tb in range(NTB):
                xt = xrp.tile([128, ND, 128], dt.float32, tag="xt", name="xt")
                nc.sync.dma_start(xt[:], xtf[tb].transpose([1, 0, 2]))
                ps = ptr.tile([128, E], dt.float32, name=f"psr_{tb}", tag="psr")
                for dc in range(ND):
                    nc.tensor.matmul(ps[:], lhsT=xt[:, dc, :], rhs=wr_sb[:, dc, :],
                                     start=(dc == 0), stop=(dc == ND - 1))
                nc.scalar.copy(lg[:, tb, :], ps[:])
                nc.vector.max(mx[:, tb, :], lg[:, tb, :])
              l1 = mx[:, :, 0]                       # [128, NTB] strided
              l2 = mx[:, :, 1]
              d12 = vec.tile([128, NTB], dt.float32, name="d12")
              p1 = vec.tile([128, NTB], dt.float32, name="p1")
              nc.vector.tensor_sub(d12[:], l1, l2)
              nc.scalar.activation(p1[:], d12[:],
                                   bass.mybir.ActivationFunctionType.Sigmoid)
              # cmb[:, tb, e] = (lg==l1)*p1 + (lg==l2)*(1-p1)
              m1 = vec.tile([128, NTB, E], dt.float32, name="m1")
              m2 = vec.tile([128, NTB, E], dt.float32, name="m2")
              l1b = l1.unsqueeze(2).to_broadcast([128, NTB, E])
              l2b = l2.unsqueeze(2).to_broadcast([128, NTB, E])
              p1b = p1[:].unsqueeze(2).to_broadcast([128, NTB, E])
              nc.vector.tensor_tensor(m1[:], lg[:], l1b, mybir.AluOpType.is_equal)
              nc.vector.tensor_tensor(m2[:], lg[:], l2b, mybir.AluOpType.is_equal)
              nc.vector.tensor_tensor(m1[:], m1[:], p1b, mybir.AluOpType.mult)
              t2 = vec.tile([128, NTB, E], dt.float32, name="t2")
              nc.vector.tensor_tensor(t2[:], m2[:], p1b, mybir.AluOpType.mult)
              nc.vector.tensor_sub(m2[:], m2[:], t2[:])
              nc.vector.tensor_add(cmb[:], m1[:], m2[:])

            # ---- FFN over experts ----
            def emit_l1(e):
                for hc in range(NH):
                    w1t = w1p.tile([128, ND, 128], dt.bfloat16, tag="w1t")
                    nc.sync.dma_start(w1t[:], w1d[e, hc])
                    for nt in range(NNT):
                        ps = pmm.tile([128, NT], dt.float32, name=f"ps1_{e}_{hc}_{nt}", tag="ps")
                        sl = slice(nt * NT, (nt + 1) * NT)
                        for dc in range(ND):
                            nc.tensor.matmul(ps[:], lhsT=w1t[:, dc, :],
                                             rhs=xall[:, dc, sl],
                                             start=(dc == 0), stop=(dc == ND - 1))
                        nc.scalar.activation(hT[:, hc, sl], ps[:],
                                             bass.mybir.ActivationFunctionType.Gelu,
                                             bias=b1sb[:, e, hc:hc + 1])
            # layer 2 + transpose + weighted accumulate
            def emit_l2(e):
                for dc in range(ND):
                    w2t = w2p.tile([128, NH, 128], dt.bfloat16, tag="w2t")
                    nc.sync.dma_start(w2t[:], w2d[e, dc])
                    for nt in range(NNT):
                        ps = pmm.tile([128, NT], dt.float32, name=f"ps2_{e}_{dc}_{nt}", tag="ps")
                        sl = slice(nt * NT, (nt + 1) * NT)
                        for hc in range(NH):
                            nc.tensor.matmul(ps[:], lhsT=w2t[:, hc, :],
                                             rhs=hT[:, hc, sl],
                                             start=(hc == 0), stop=(hc == NH - 1))
                        yt = vec.tile([128, NT], dt.bfloat16, tag="yt")
                        nc.scalar.activation(yt[:], ps[:],
                                             bass.mybir.ActivationFunctionType.Identity,
                                             bias=b2sb[:, e, dc:dc + 1])
                        for tc_ in range(NT // 128):
                            tb = nt * (NT // 128) + tc_
                            pt = ptr.tile([128, 128], dt.bfloat16,
                                          name=f"pt_{e}_{dc}_{nt}_{tc_}", tag="pt")
                            nc.tensor.transpose(
                                pt[:], yt[:, tc_ * 128:(tc_ + 1) * 128], identb[:])
                            a_sl = acc[:, tb, dc * 128:(dc + 1) * 128]
                            nc.vector.scalar_tensor_tensor(
                                a_sl, pt[:], cmb[:, tb, e:e + 1], a_sl,
                                op0=mybir.AluOpType.mult,
                                op1=mybir.AluOpType.add)

            emit_l1(0)
            emit_router()
            emit_l2(0)
            for e in range(1, E):
                emit_l1(e)
                emit_l2(e)

            for tb in range(NTB):
                nc.sync.dma_start(out[tb * 128:(tb + 1) * 128, :], acc[:, tb, :])

    nc.compile()
    return nc


def _prep_inputs(x, Wr, W1, b1, W2, b2):
    x = np.ascontiguousarray(np.asarray(x, np.float32)).reshape(T, DIM)
    Wr = np.ascontiguousarray(np.asarray(Wr, np.float32))
    W1 = np.asarray(W1, np.float32)
    b1 = np.asarray(b1, np.float32)
    W2 = np.asarray(W2, np.float32)
    b2 = np.asarray(b2, np.float32)

    w1d = np.ascontiguousarray(
        W1.astype(BF16).reshape(E, ND, 128, NH, 128).transpose(0, 3, 2, 1, 4))
    w2d = np.ascontiguousarray(
        W2.astype(BF16).reshape(E, NH, 128, ND, 128).transpose(0, 3, 2, 1, 4))
    b1d = np.ascontiguousarray(b1.reshape(E, NH, 128).transpose(2, 0, 1))
    b2d = np.ascontiguousarray(b2.reshape(E, ND, 128).transpose(2, 0, 1))

    in_maps = []
    for c in range(N_CORES):
        xs = x[c * TPC:(c + 1) * TPC]                      # [1024, 1024]
        # [NTB, ND, 128 d, 128 t] tiles of x^T
        xt = np.ascontiguousarray(
            xs.reshape(NTB, 128, ND, 128).transpose(0, 2, 3, 1))
        in_maps.append({
            "xtf": xt,
            "xbt": xt.astype(BF16),
            "wr": Wr,
            "w1d": w1d,
            "w2d": w2d,
            "b1d": b1d,
            "b2d": b2d,
        })
    return in_maps


def kernel(x, Wr, W1, b1, W2, b2, _profile=None):
    global _compiled
    from concourse.bass_utils import run_bass_kernel_spmd

    if _compiled is None:
        _compiled = _build()
    nc = _compiled
    in_maps = _prep_inputs(x, Wr, W1, b1, W2, b2)
    kwargs = {}
    if _profile:
        kwargs = dict(trace=True, tmpdir=_profile)
    res = run_bass_kernel_spmd(nc, in_maps, core_ids=list(range(N_CORES)), **kwargs)
    shards = [res.results[c]["out_shard"] for c in range(N_CORES)]
    full = np.concatenate(shards, axis=0).reshape(4, 2048, DIM).astype(np.float32)
    if _profile:
        return full, res
    return full
